# revision 6
# baseline (speedup 1.0000x reference)
"""Trainium2 Bass kernel for nn_DensityMap (histogram_binning).

Reference computation (B=64, V=2048, G=256, sigma=2, target=1):
  - soft-rectangle splat per macro: density[b,y,x] = sum_v m_v * yin[b,v,y] * xin[b,v,x]
    with {x,y}in = sigmoid(grid_size - 2*|coord - grid_pos|)
  - 13x13 Gaussian smoothing (separable, reflect padding)
  - overflow_loss = mean(relu(density - 1)^2)

Device strategy (8 NeuronCores, data-parallel over batch, 8 batches/core):
  - Host: compact macros by mask (mask is batch-independent) -> V' active
    macros padded to a multiple of 128; padded macros get grid_pos=-1e6,
    grid_size=0 so their sigmoid underflows to exactly 0.
  - Profiles in [v(partition), g(free)] layout, per 128-macro chunk:
      DVE  tensor_scalar(op0=subtract, op1=abs_max, s2=0) -> |coords - pos|
      ACT  sigmoid(-2*absd + grid_size)   (bias is per-partition = per-macro)
  - Splat: D[y,x] = sum_v yin[v,y] * xin[v,x] = K-accumulated matmuls
    (lhsT = yin chunk slice, rhs = xin chunk slice) into PSUM.
  - Smoothing Z = Sm @ D @ Sm^T computed as two "transpose-by-matmul" stages
    against the constant SmT = Sm.T (reflect padding folded into Sm):
      W[x,j] = sum_y D[y,x] * SmT[y,j]      (lhsT = D tile,  rhs = SmT)
      Z[i,j] = sum_x W[x,i] * SmT[x,j]      (lhsT = W tile,  rhs = SmT)
    Z lands directly in natural [y,x] orientation, no transposes needed.
  - Loss: t = relu(Z-1) on DVE, then ACT Square with accum_out -> per-tile
    partial sums; host reduces across tiles/cores.
"""

import numpy as np

GRID = 256
SIGMA = 2.0
B = 64
V = 2048
NCORES = 8
B_LOC = B // NCORES  # 8 batches per core
SIG_SPLIT = 2        # sigmoid ops per (chunk, axis): each covers B_LOC/SIG_SPLIT batches

_prog_cache = {}


def _gaussian_sep_f32():
    """1-D normalized Gaussian taps, f32 math to mirror the reference."""
    k_size = int(6 * SIGMA) | 1  # 13
    x = (np.arange(k_size, dtype=np.float32) - k_size // 2).astype(np.float32)
    k1 = np.exp(-(x ** 2) / np.float32(2.0 * SIGMA ** 2)).astype(np.float32)
    # reference normalizes the 2-D outer product by its sum; that equals
    # outer(k1/k1.sum(), k1/k1.sum()) exactly in real arithmetic.
    k1n = (k1 / k1.sum()).astype(np.float32)
    return k1n, k_size


def _smoothing_matrix():
    """Sm[i, g]: weight of input pixel g for output pixel i (reflect pad)."""
    k1n, k_size = _gaussian_sep_f32()
    pad = k_size // 2
    G = GRID
    Sm = np.zeros((G, G), dtype=np.float32)
    for i in range(G):
        for t in range(-pad, pad + 1):
            g = i + t
            if g < 0:
                g = -g
            elif g > G - 1:
                g = 2 * (G - 1) - g
            Sm[i, g] += k1n[t + pad]
    return Sm


MM_FP32R = False  # use float32r (1 cyc/row vs 4) for all matmuls


def _build_program(kch, mm_r=None):
    import concourse.tile as tile
    from concourse import bacc, mybir
    from contextlib import ExitStack

    if mm_r is None:
        mm_r = MM_FP32R
    F32 = mybir.dt.float32
    AF = mybir.ActivationFunctionType
    OP = mybir.AluOpType

    def mm_cast(ap):
        return ap.bitcast(mybir.dt.float32r) if mm_r else ap

    nc = bacc.Bacc("TRN2", target_bir_lowering=False, debug=False)

    pos_d = nc.dram_tensor("pos", [128, kch, B_LOC, 2], F32, kind="ExternalInput")
    gs_d = nc.dram_tensor("gs", [128, kch, 2], F32, kind="ExternalInput")
    coords_d = nc.dram_tensor("coords", [128, GRID], F32, kind="ExternalInput")
    smt_d = nc.dram_tensor("smt", [2, 128, GRID], F32, kind="ExternalInput")
    dens_d = nc.dram_tensor("dens", [B_LOC, GRID, GRID], F32, kind="ExternalOutput")
    lossp_d = nc.dram_tensor("lossp", [128, 2 * B_LOC], F32, kind="ExternalOutput")

    dens_ap = dens_d.ap()
    bpg = B_LOC // SIG_SPLIT  # batches per sigmoid op

    with ExitStack() as ctx:
        tc = ctx.enter_context(tile.TileContext(nc))
        consts = ctx.enter_context(tc.tile_pool(name="consts", bufs=1))
        profs = ctx.enter_context(tc.tile_pool(name="profs", bufs=1))
        scratch = ctx.enter_context(tc.tile_pool(name="scratch", bufs=2))
        sbwork = ctx.enter_context(tc.tile_pool(name="sbwork", bufs=2))
        ps_d = ctx.enter_context(tc.tile_pool(name="psd", bufs=1, space="PSUM"))
        ps_w = ctx.enter_context(tc.tile_pool(name="psw", bufs=1, space="PSUM"))
        ps_z = ctx.enter_context(tc.tile_pool(name="psz", bufs=2, space="PSUM"))

        coords = consts.tile([128, GRID], F32)
        nc.sync.dma_start(coords[:], coords_d[:])
        smt = consts.tile([128, 2, GRID], F32)
        for kc in range(2):
            nc.sync.dma_start(smt[:, kc, :], smt_d.ap()[kc])
        pos_t = consts.tile([128, kch, B_LOC, 2], F32)
        nc.sync.dma_start(pos_t[:], pos_d[:])
        gs_t = consts.tile([128, kch, 2], F32)
        nc.sync.dma_start(gs_t[:], gs_d[:])
        lacc = consts.tile([128, 2 * B_LOC], F32)

        # ---- profiles: xin/yin for every (chunk, axis), all batches ----
        prof = {}  # (k, axis) -> [128, B_LOC, GRID] tile
        for k in range(kch):
            for a in (0, 1):  # 0 = x-axis, 1 = y-axis
                p = profs.tile([128, B_LOC, GRID], F32, tag=f"prof_{k}_{a}")
                absd = scratch.tile([128, B_LOC, GRID], F32, tag="absd")
                for b in range(B_LOC):
                    nc.vector.tensor_scalar(
                        out=absd[:, b, :], in0=coords[:],
                        scalar1=pos_t[:, k, b, a:a + 1], scalar2=None,
                        op0=OP.subtract,
                    )
                I32 = mybir.dt.int32
                for b in range(B_LOC):
                    nc.vector.tensor_scalar(
                        out=absd[:, b, :].bitcast(I32),
                        in0=absd[:, b, :].bitcast(I32),
                        scalar1=0x7FFFFFFF, scalar2=None,
                        op0=OP.bitwise_and,
                    )
                for g in range(SIG_SPLIT):
                    nc.scalar.activation(
                        out=p[:, g * bpg:(g + 1) * bpg, :],
                        in_=absd[:, g * bpg:(g + 1) * bpg, :],
                        func=AF.Sigmoid,
                        bias=gs_t[:, k, a:a + 1], scale=-2.0,
                    )
                prof[(k, a)] = p

        # ---- per-batch: splat -> smooth -> output + loss ----
        for b in range(B_LOC):
            # splat: D = yin^T @ xin, [y, x] layout, two 128-row halves
            d_sb = sbwork.tile([128, 2, GRID], F32, tag="d_sb")
            for my in (0, 1):
                dm = ps_d.tile([128, GRID], F32, tag=f"d{my}")
                for k in range(kch):
                    nc.tensor.matmul(
                        dm[:],
                        mm_cast(prof[(k, 1)][:, b, my * 128:(my + 1) * 128]),
                        mm_cast(prof[(k, 0)][:, b, :]),
                        start=(k == 0), stop=(k == kch - 1),
                    )
                if my == 0:
                    nc.scalar.copy(d_sb[:, my, :], dm[:])
                else:
                    nc.vector.tensor_copy(d_sb[:, my, :], dm[:])

            # stage 1: W[x,j] = sum_y D[y,x] * SmT[y,j]
            w_sb = sbwork.tile([128, 2, GRID], F32, tag="w_sb")
            for mx in (0, 1):
                wm = ps_w.tile([128, GRID], F32, tag=f"w{mx}")
                for my in (0, 1):
                    nc.tensor.matmul(
                        wm[:],
                        mm_cast(d_sb[:, my, mx * 128:(mx + 1) * 128]),
                        mm_cast(smt[:, my, :]),
                        start=(my == 0), stop=(my == 1),
                    )
                if mx == 0:
                    nc.scalar.copy(w_sb[:, mx, :], wm[:])
                else:
                    nc.vector.tensor_copy(w_sb[:, mx, :], wm[:])

            # stage 2: Z[i,j] = sum_x W[x,i] * SmT[x,j]  (final density)
            for ma in (0, 1):
                zm = ps_z.tile([128, GRID], F32, tag=f"z{ma}")
                for mx in (0, 1):
                    nc.tensor.matmul(
                        zm[:],
                        mm_cast(w_sb[:, mx, ma * 128:(ma + 1) * 128]),
                        mm_cast(smt[:, mx, :]),
                        start=(mx == 0), stop=(mx == 1),
                    )
                z_sb = sbwork.tile([128, GRID], F32, tag=f"zsb{ma}")
                if ma == 0:
                    nc.scalar.copy(z_sb[:], zm[:])
                else:
                    nc.vector.tensor_copy(z_sb[:], zm[:])
                nc.sync.dma_start(dens_ap[b, ma * 128:(ma + 1) * 128, :], z_sb[:])

                # loss partial: sum(relu(z-1)^2) along free dim
                t = scratch.tile([128, GRID], F32, tag="relu")
                nc.vector.tensor_scalar(
                    out=t[:], in0=z_sb[:], scalar1=1.0, scalar2=0.0,
                    op0=OP.subtract, op1=OP.max,
                )
                sqd = scratch.tile([128, GRID], F32, tag="sqd")
                col = b * 2 + ma
                nc.scalar.activation(
                    out=sqd[:], in_=t[:], func=AF.Square,
                    accum_out=lacc[:, col:col + 1],
                )

        nc.sync.dma_start(lossp_d[:], lacc[:])

    nc.compile()
    return nc


def _prepare(positions, sizes, macro_mask):
    """Host-side sharding/compaction. Returns (kch, in_maps)."""
    positions = np.asarray(positions, dtype=np.float32)
    sizes = np.asarray(sizes, dtype=np.float32)
    macro_mask = np.asarray(macro_mask)

    G = GRID
    # host-side macro compaction: mask is shared across batches
    keep = np.nonzero(macro_mask)[0]
    vk = len(keep)
    kch = max(1, (vk + 127) // 128)
    vp = kch * 128

    grid_pos = ((positions + np.float32(1.0)) / np.float32(2.0)
                * np.float32(G - 1)).astype(np.float32)      # (B, V, 2)
    grid_sizes = (sizes * np.float32(G / 2.0)).astype(np.float32)  # (V, 2)

    # compacted + padded per-macro data
    pos_c = np.full((B, vp, 2), -1.0e6, dtype=np.float32)
    pos_c[:, :vk, :] = grid_pos[:, keep, :]
    gs_c = np.zeros((vp, 2), dtype=np.float32)
    gs_c[:vk, :] = grid_sizes[keep, :]

    # device layouts
    # pos: (128, kch, B_LOC, 2) per core;  gs: (128, kch, 2) shared
    pos_r = pos_c.reshape(B, kch, 128, 2)                     # [b, k, v, a]
    gs_r = gs_c.reshape(kch, 128, 2).transpose(1, 0, 2).copy()  # [v, k, a]

    coords_np = np.tile(np.arange(G, dtype=np.float32), (128, 1)).copy()
    smt_np = np.ascontiguousarray(
        _smoothing_matrix().T.reshape(2, 128, G))             # SmT k-chunks

    in_maps = []
    for c in range(NCORES):
        bsl = slice(c * B_LOC, (c + 1) * B_LOC)
        pos_core = np.ascontiguousarray(
            pos_r[bsl].transpose(2, 1, 0, 3))                 # (128, kch, B_LOC, 2)
        in_maps.append({
            "pos": pos_core,
            "gs": gs_r,
            "coords": coords_np,
            "smt": smt_np,
        })
    return kch, in_maps


def _postprocess(results):
    G = GRID
    dens = np.concatenate([r["dens"] for r in results], axis=0)  # (B, G, G)
    density = dens[:, None, :, :].astype(np.float32)
    loss_total = np.sum(
        [r["lossp"].astype(np.float64).sum() for r in results])
    overflow_loss = np.float32(loss_total / (B * G * G))
    return density, overflow_loss


def get_program(kch):
    key = (kch, MM_FP32R)
    if key not in _prog_cache:
        _prog_cache[key] = _build_program(kch)
    return _prog_cache[key]


def kernel(positions, sizes, macro_mask):
    from concourse.bass_utils import run_bass_kernel_spmd

    kch, in_maps = _prepare(positions, sizes, macro_mask)
    nc = get_program(kch)
    res = run_bass_kernel_spmd(nc, in_maps, core_ids=list(range(NCORES)))
    return _postprocess(res.results)


# revision 7
# speedup vs baseline: 1.3594x; 1.3594x over previous
"""Trainium2 Bass kernel for nn_DensityMap (histogram_binning).

Reference computation (B=64, V=2048, G=256, sigma=2, target=1):
  - soft-rectangle splat per macro: density[b,y,x] = sum_v m_v * yin[b,v,y] * xin[b,v,x]
    with {x,y}in = sigmoid(grid_size - 2*|coord - grid_pos|)
  - 13x13 Gaussian smoothing (separable, reflect padding)
  - overflow_loss = mean(relu(density - 1)^2)

Device strategy (8 NeuronCores, data-parallel over batch, 8 batches/core):
  - Host: compact macros by mask (mask is batch-independent) -> V' active
    macros padded to a multiple of 128; padded macros get grid_pos=-1e6,
    grid_size=0 so their sigmoid underflows to exactly 0.
  - Profiles in [v(partition), g(free)] layout, per 128-macro chunk:
    |coords-pos| via either DVE (subtract, then bitwise-AND sign clear) or
    ACT (Abs activation with bias=-pos), split for engine balance; then one
    big ACT sigmoid(-2*absd + grid_size) per (chunk, axis) spanning batches
    (bias is per-partition = per-macro, batch-independent).
  - All matmuls in float32r (1 cyc/row vs 4 for fp32; rel err ~2e-5): every
    matmul operand is produced by an engine op writing a float32r tile
    (walrus requires "rounded" producers; DMA-fed f32r crashes the PE).
  - Splat: D[y,x] = sum_v yin[v,y] * xin[v,x] = K-accumulated matmuls.
  - Smoothing Z = Sm @ D @ Sm^T as two "transpose-by-matmul" stages against
    the constant SmT = Sm.T (reflect padding folded in):
      W[x,j] = sum_y D[y,x] * SmT[y,j]      (lhsT = D tile,  rhs = SmT)
      Z[i,j] = sum_x W[x,i] * SmT[x,j]      (lhsT = W tile,  rhs = SmT)
    Z lands directly in natural [y,x] orientation, no transposes needed.
  - Loss fully on DVE: t = relu(z-1) via tensor_scalar(sub, max); then
    scalar_tensor_tensor(t*t) with accum_out -> per-tile partial sums;
    host reduces across tiles/cores.
"""

import numpy as np

GRID = 256
SIGMA = 2.0
B = 64
V = 2048
NCORES = 8
B_LOC = B // NCORES  # 8 batches per core
SIG_SPLIT = 2        # sigmoid ops per (chunk, axis)
ACT_ABS_GROUPS = 6   # of the kch*2 (chunk, axis) groups, how many use ACT Abs

_prog_cache = {}


def _gaussian_sep_f32():
    """1-D normalized Gaussian taps, f32 math to mirror the reference."""
    k_size = int(6 * SIGMA) | 1  # 13
    x = (np.arange(k_size, dtype=np.float32) - k_size // 2).astype(np.float32)
    k1 = np.exp(-(x ** 2) / np.float32(2.0 * SIGMA ** 2)).astype(np.float32)
    k1n = (k1 / k1.sum()).astype(np.float32)
    return k1n, k_size


def _smoothing_matrix():
    """Sm[i, g]: weight of input pixel g for output pixel i (reflect pad)."""
    k1n, k_size = _gaussian_sep_f32()
    pad = k_size // 2
    G = GRID
    Sm = np.zeros((G, G), dtype=np.float32)
    for i in range(G):
        for t in range(-pad, pad + 1):
            g = i + t
            if g < 0:
                g = -g
            elif g > G - 1:
                g = 2 * (G - 1) - g
            Sm[i, g] += k1n[t + pad]
    return Sm


def _build_program(kch):
    import concourse.tile as tile
    from concourse import bacc, mybir
    from contextlib import ExitStack

    F32 = mybir.dt.float32
    F32R = mybir.dt.float32r
    I32 = mybir.dt.int32
    AF = mybir.ActivationFunctionType
    OP = mybir.AluOpType

    nc = bacc.Bacc("TRN2", target_bir_lowering=False, debug=False)

    # pos last dim: 0,1 = +pos (x,y); 2,3 = -pos (x,y)
    pos_d = nc.dram_tensor("pos", [128, kch, B_LOC, 4], F32, kind="ExternalInput")
    gs_d = nc.dram_tensor("gs", [128, kch, 2], F32, kind="ExternalInput")
    coords_d = nc.dram_tensor("coords", [128, GRID], F32, kind="ExternalInput")
    smt_d = nc.dram_tensor("smt", [2, 128, GRID], F32, kind="ExternalInput")
    dens_d = nc.dram_tensor("dens", [B_LOC, GRID, GRID], F32, kind="ExternalOutput")
    lossp_d = nc.dram_tensor("lossp", [128, 2 * B_LOC], F32, kind="ExternalOutput")

    dens_ap = dens_d.ap()
    bpg = B_LOC // SIG_SPLIT

    with ExitStack() as ctx:
        tc = ctx.enter_context(tile.TileContext(nc))
        consts = ctx.enter_context(tc.tile_pool(name="consts", bufs=1))
        profs = ctx.enter_context(tc.tile_pool(name="profs", bufs=1))
        scratch = ctx.enter_context(tc.tile_pool(name="scratch", bufs=2))
        sbwork = ctx.enter_context(tc.tile_pool(name="sbwork", bufs=2))
        ps_d = ctx.enter_context(tc.tile_pool(name="psd", bufs=1, space="PSUM"))
        ps_w = ctx.enter_context(tc.tile_pool(name="psw", bufs=1, space="PSUM"))
        ps_z = ctx.enter_context(tc.tile_pool(name="psz", bufs=2, space="PSUM"))

        coords = consts.tile([128, GRID], F32)
        nc.sync.dma_start(coords[:], coords_d[:])
        smt_f32 = consts.tile([128, 2, GRID], F32)
        for kc in range(2):
            nc.sync.dma_start(smt_f32[:, kc, :], smt_d.ap()[kc])
        # rounded f32r copy of the smoothing matrix (matmul operand)
        smt = consts.tile([128, 2, GRID], F32R)
        nc.vector.tensor_copy(smt[:], smt_f32[:])
        pos_t = consts.tile([128, kch, B_LOC, 4], F32)
        nc.sync.dma_start(pos_t[:], pos_d[:])
        gs_t = consts.tile([128, kch, 2], F32)
        nc.sync.dma_start(gs_t[:], gs_d[:])
        lacc = consts.tile([128, 2 * B_LOC], F32)

        # ---- profiles: xin/yin for every (chunk, axis), all batches ----
        prof = {}
        group_idx = 0
        for k in range(kch):
            for a in (0, 1):  # 0 = x-axis, 1 = y-axis
                p = profs.tile([128, B_LOC, GRID], F32R, tag=f"prof_{k}_{a}")
                absd = scratch.tile([128, B_LOC, GRID], F32, tag="absd")
                use_act = group_idx < ACT_ABS_GROUPS
                group_idx += 1
                if use_act:
                    for b in range(B_LOC):
                        nc.scalar.activation(
                            out=absd[:, b, :], in_=coords[:], func=AF.Abs,
                            bias=pos_t[:, k, b, (a + 2):(a + 3)], scale=1.0,
                        )
                else:
                    for b in range(B_LOC):
                        nc.vector.tensor_scalar(
                            out=absd[:, b, :], in0=coords[:],
                            scalar1=pos_t[:, k, b, a:a + 1], scalar2=None,
                            op0=OP.subtract,
                        )
                    for b in range(B_LOC):
                        nc.vector.tensor_scalar(
                            out=absd[:, b, :].bitcast(I32),
                            in0=absd[:, b, :].bitcast(I32),
                            scalar1=0x7FFFFFFF, scalar2=None,
                            op0=OP.bitwise_and,
                        )
                for g in range(SIG_SPLIT):
                    nc.scalar.activation(
                        out=p[:, g * bpg:(g + 1) * bpg, :],
                        in_=absd[:, g * bpg:(g + 1) * bpg, :],
                        func=AF.Sigmoid,
                        bias=gs_t[:, k, a:a + 1], scale=-2.0,
                    )
                prof[(k, a)] = p

        # ---- per-batch: splat -> smooth -> output + loss ----
        for b in range(B_LOC):
            d_sb = sbwork.tile([128, 2, GRID], F32R, tag="d_sb")
            for my in (0, 1):
                dm = ps_d.tile([128, GRID], F32, tag=f"d{my}")
                for k in range(kch):
                    nc.tensor.matmul(
                        dm[:],
                        prof[(k, 1)][:, b, my * 128:(my + 1) * 128],
                        prof[(k, 0)][:, b, :],
                        start=(k == 0), stop=(k == kch - 1),
                    )
                if my == 0:
                    nc.scalar.copy(d_sb[:, my, :], dm[:])
                else:
                    nc.vector.tensor_copy(d_sb[:, my, :], dm[:])

            w_sb = sbwork.tile([128, 2, GRID], F32R, tag="w_sb")
            for mx in (0, 1):
                wm = ps_w.tile([128, GRID], F32, tag=f"w{mx}")
                for my in (0, 1):
                    nc.tensor.matmul(
                        wm[:],
                        d_sb[:, my, mx * 128:(mx + 1) * 128],
                        smt[:, my, :],
                        start=(my == 0), stop=(my == 1),
                    )
                if mx == 0:
                    nc.scalar.copy(w_sb[:, mx, :], wm[:])
                else:
                    nc.vector.tensor_copy(w_sb[:, mx, :], wm[:])

            for ma in (0, 1):
                zm = ps_z.tile([128, GRID], F32, tag=f"z{ma}")
                for mx in (0, 1):
                    nc.tensor.matmul(
                        zm[:],
                        w_sb[:, mx, ma * 128:(ma + 1) * 128],
                        smt[:, mx, :],
                        start=(mx == 0), stop=(mx == 1),
                    )
                z_sb = sbwork.tile([128, GRID], F32, tag=f"zsb{ma}")
                if ma == 0:
                    nc.scalar.copy(z_sb[:], zm[:])
                else:
                    nc.vector.tensor_copy(z_sb[:], zm[:])
                nc.sync.dma_start(dens_ap[b, ma * 128:(ma + 1) * 128, :], z_sb[:])

                # loss partial on DVE: t = relu(z-1); lacc_col = sum(t*t)
                t = scratch.tile([128, GRID], F32, tag="relu")
                nc.vector.tensor_scalar(
                    out=t[:], in0=z_sb[:], scalar1=1.0, scalar2=0.0,
                    op0=OP.subtract, op1=OP.max,
                )
                sqd = scratch.tile([128, GRID], F32, tag="sqd")
                col = b * 2 + ma
                nc.vector.scalar_tensor_tensor(
                    out=sqd[:], in0=t[:], scalar=0.0, in1=t[:],
                    op0=OP.add, op1=OP.mult,
                    accum_out=lacc[:, col:col + 1],
                )

        nc.sync.dma_start(lossp_d[:], lacc[:])

    nc.compile()
    return nc


def _prepare(positions, sizes, macro_mask):
    """Host-side sharding/compaction. Returns (kch, in_maps)."""
    positions = np.asarray(positions, dtype=np.float32)
    sizes = np.asarray(sizes, dtype=np.float32)
    macro_mask = np.asarray(macro_mask)

    G = GRID
    keep = np.nonzero(macro_mask)[0]
    vk = len(keep)
    kch = max(1, (vk + 127) // 128)
    vp = kch * 128

    grid_pos = ((positions + np.float32(1.0)) / np.float32(2.0)
                * np.float32(G - 1)).astype(np.float32)      # (B, V, 2)
    grid_sizes = (sizes * np.float32(G / 2.0)).astype(np.float32)  # (V, 2)

    pos_c = np.full((B, vp, 2), -1.0e6, dtype=np.float32)
    pos_c[:, :vk, :] = grid_pos[:, keep, :]
    gs_c = np.zeros((vp, 2), dtype=np.float32)
    gs_c[:vk, :] = grid_sizes[keep, :]

    pos_r = pos_c.reshape(B, kch, 128, 2)                     # [b, k, v, a]
    gs_r = gs_c.reshape(kch, 128, 2).transpose(1, 0, 2).copy()  # [v, k, a]

    coords_np = np.tile(np.arange(G, dtype=np.float32), (128, 1)).copy()
    smt_np = np.ascontiguousarray(
        _smoothing_matrix().T.reshape(2, 128, G))

    in_maps = []
    for c in range(NCORES):
        bsl = slice(c * B_LOC, (c + 1) * B_LOC)
        pc = pos_r[bsl].transpose(2, 1, 0, 3)                 # (128, kch, B_LOC, 2)
        pos_core = np.concatenate([pc, -pc], axis=3)          # (.., 4): +pos, -pos
        in_maps.append({
            "pos": np.ascontiguousarray(pos_core),
            "gs": gs_r,
            "coords": coords_np,
            "smt": smt_np,
        })
    return kch, in_maps


def _postprocess(results):
    G = GRID
    dens = np.concatenate([r["dens"] for r in results], axis=0)  # (B, G, G)
    density = dens[:, None, :, :].astype(np.float32)
    loss_total = np.sum(
        [r["lossp"].astype(np.float64).sum() for r in results])
    overflow_loss = np.float32(loss_total / (B * G * G))
    return density, overflow_loss


def get_program(kch):
    if kch not in _prog_cache:
        _prog_cache[kch] = _build_program(kch)
    return _prog_cache[kch]


def kernel(positions, sizes, macro_mask):
    from concourse.bass_utils import run_bass_kernel_spmd

    kch, in_maps = _prepare(positions, sizes, macro_mask)
    nc = get_program(kch)
    res = run_bass_kernel_spmd(nc, in_maps, core_ids=list(range(NCORES)))
    return _postprocess(res.results)


# revision 14
# speedup vs baseline: 1.8285x; 1.3451x over previous
"""Trainium2 Bass kernel for nn_DensityMap (histogram_binning).

Reference computation (B=64, V=2048, G=256, sigma=2, target=1):
  - soft-rectangle splat per macro: density[b,y,x] = sum_v m_v * yin[b,v,y] * xin[b,v,x]
    with {x,y}in = sigmoid(grid_size - 2*|coord - grid_pos|)
  - 13x13 Gaussian smoothing (separable, reflect padding)
  - overflow_loss = mean(relu(density - 1)^2)

Device strategy (8 NeuronCores, data-parallel over batch, 8 batches/core):
  - Host: compact macros by mask (mask is batch-independent) -> V' active
    macros padded to a multiple of 128; padded macros get grid_pos=-60000,
    grid_size=0 so their sigmoid underflows to exactly 0.
  - |coords - pos| is built per (chunk, axis, batch) unit in [v, g] layout by
    one of three engine paths (chosen per (chunk, axis) group for balance):
      PE:  d = coords - pos as a K=3 fp16 matmul (rows: coords*1, -p_hi*1,
           -p_lo*1; exact split keeps error ~3e-5) into PSUM, then one DVE
           bitwise-AND (sign clear) per pair of units -> |d| in SBUF
      DVE: tensor_scalar subtract per unit + one merged bitwise-AND per group
      ACT: Abs activation with per-partition bias = -pos
  - One big ACT sigmoid(-2*|d| + grid_size) per (chunk, axis) spanning all
    batches (bias is per-partition = per-macro, batch-independent).
  - All density matmuls in float32r (1 cyc/row vs 4 for fp32; rel ~2e-5).
    Every f32r operand is produced by an engine op (walrus "rounded" rule).
  - Splat: D[y,x] = sum_v yin[v,y]*xin[v,x], K-accumulated; batches 0-3
    accumulate chunk-by-chunk DURING profile generation (PSUM: 4 sub banks +
    4 wave-A banks), batches 4-7 splat afterwards.
  - Smoothing Z = Sm @ D @ Sm^T as two transpose-by-matmul stages against
    constant SmT (reflect padding folded in); Z lands in natural [y,x].
  - Loss on DVE: relu(z-1) then squared-sum via scalar_tensor_tensor
    accum_out; host reduces partials.
"""

import numpy as np

GRID = 256
SIGMA = 2.0
B = 64
V = 2048
NCORES = 8
B_LOC = B // NCORES
WAVE = 2             # batches that splat-accumulate during profile phase
PAD_POS = -60000.0   # fits fp16; sigmoid underflows to exactly 0
ACT_ABS_GROUPS = 0   # (chunk,axis) groups using ACT Abs
DVE_SUB_GROUPS = 5   # groups using DVE subtract; rest use PE fp16 matmul

_prog_cache = {}


def _gaussian_sep_f32():
    k_size = int(6 * SIGMA) | 1  # 13
    x = (np.arange(k_size, dtype=np.float32) - k_size // 2).astype(np.float32)
    k1 = np.exp(-(x ** 2) / np.float32(2.0 * SIGMA ** 2)).astype(np.float32)
    return (k1 / k1.sum()).astype(np.float32), k_size


def _smoothing_matrix():
    k1n, k_size = _gaussian_sep_f32()
    pad = k_size // 2
    G = GRID
    Sm = np.zeros((G, G), dtype=np.float32)
    for i in range(G):
        for t in range(-pad, pad + 1):
            g = i + t
            if g < 0:
                g = -g
            elif g > G - 1:
                g = 2 * (G - 1) - g
            Sm[i, g] += k1n[t + pad]
    return Sm


def _group_paths(kch):
    """Assign each of the kch*2 (chunk, axis) groups an abs path, spreading
    the non-PE groups evenly for pipeline balance."""
    n = kch * 2
    paths = ["pe"] * n
    special = (["act"] * ACT_ABS_GROUPS) + (["dve"] * DVE_SUB_GROUPS)
    if special:
        step = max(1, n // len(special))
        i = 0
        for s in special:
            while i < n and paths[i] != "pe":
                i += 1
            if i >= n:
                break
            paths[i] = s
            i += step
    return paths


def _build_program(kch):
    import concourse.tile as tile
    from concourse import bacc, mybir
    from contextlib import ExitStack

    F32 = mybir.dt.float32
    F32R = mybir.dt.float32r
    F16 = mybir.dt.float16
    I32 = mybir.dt.int32
    AF = mybir.ActivationFunctionType
    OP = mybir.AluOpType

    nc = bacc.Bacc("TRN2", target_bir_lowering=False, debug=False)

    pos_d = nc.dram_tensor("pos", [128, kch, B_LOC, 4], F32, kind="ExternalInput")
    gs_d = nc.dram_tensor("gs", [128, kch, 2], F32, kind="ExternalInput")
    coords_d = nc.dram_tensor("coords", [128, GRID], F32, kind="ExternalInput")
    smt_d = nc.dram_tensor("smt", [2, 128, GRID], F32, kind="ExternalInput")
    # fp16 K=3 subtract operands: lhsT rows [ones, -p_hi, -p_lo] per (k,b,a)
    posl_d = nc.dram_tensor("posl", [3, kch, B_LOC, 2, 128], F16,
                            kind="ExternalInput")
    cr_d = nc.dram_tensor("cr", [3, GRID], F16, kind="ExternalInput")
    dens_d = nc.dram_tensor("dens", [B_LOC, GRID, GRID], F32, kind="ExternalOutput")
    lossp_d = nc.dram_tensor("lossp", [128, B_LOC], F32, kind="ExternalOutput")

    dens_ap = dens_d.ap()
    paths = _group_paths(kch)

    with ExitStack() as ctx:
        tc = ctx.enter_context(tile.TileContext(nc))
        consts = ctx.enter_context(tc.tile_pool(name="consts", bufs=1))
        profs = ctx.enter_context(tc.tile_pool(name="profs", bufs=1))
        scratch = ctx.enter_context(tc.tile_pool(name="scratch", bufs=2))
        sbwork = ctx.enter_context(tc.tile_pool(name="sbwork", bufs=2))

        coords = consts.tile([128, GRID], F32)
        nc.sync.dma_start(coords[:], coords_d[:])
        smt_f32 = consts.tile([128, 2, GRID], F32)
        for kc in range(2):
            nc.sync.dma_start(smt_f32[:, kc, :], smt_d.ap()[kc])
        smt = consts.tile([128, 2, GRID], F32R)
        nc.vector.tensor_copy(smt[:], smt_f32[:])
        pos_t = consts.tile([128, kch, B_LOC, 4], F32)
        nc.sync.dma_start(pos_t[:], pos_d[:])
        gs_t = consts.tile([128, kch, 2], F32)
        nc.sync.dma_start(gs_t[:], gs_d[:])
        cr = consts.tile([3, GRID], F16)
        nc.sync.dma_start(cr[:], cr_d[:])
        lacc = consts.tile([128, B_LOC], F32)
        poslp = ctx.enter_context(tc.tile_pool(name="poslp", bufs=2))

        prof = {}
        d_sbA = [sbwork.tile([128, 2, GRID], F32R, tag=f"d_sbA{b}",
                             name=f"d_sbA{b}") for b in range(WAVE)]

        with (
            tc.tile_pool(name="psub", bufs=4, space="PSUM") as ps_sub,
            tc.tile_pool(name="pdA", bufs=1, space="PSUM") as ps_dA,
        ):
            dA = {}
            for b in range(WAVE):
                for my in (0, 1):
                    dA[(b, my)] = ps_dA.tile([128, GRID], F32,
                                             tag=f"dA{b}_{my}",
                                             name=f"dA{b}_{my}")

            for k in range(kch):
                need_pe = any(paths[k * 2 + a] == "pe" for a in (0, 1))
                if need_pe:
                    posl = poslp.tile([3, B_LOC, 2, 128], F16, tag="posl",
                                      name=f"posl{k}")
                    nc.sync.dma_start(posl[:], posl_d.ap()[:, k])
                for a in (0, 1):
                    gi = k * 2 + a
                    path = paths[gi]
                    p = profs.tile([128, B_LOC, GRID], F16, tag=f"prof_{k}_{a}")
                    absd = scratch.tile([128, B_LOC, GRID], F32, tag="absd")
                    if path == "pe":
                        for pr in range(B_LOC // 2):
                            sps = ps_sub.tile([128, 2, GRID], F32, tag="sub")
                            for h in (0, 1):
                                b = pr * 2 + h
                                nc.tensor.matmul(
                                    sps[:, h, :],
                                    posl[:, b, a, :],
                                    cr[:],
                                    start=True, stop=True,
                                )
                            nc.vector.tensor_scalar(
                                out=absd[:, pr * 2:pr * 2 + 2, :].bitcast(I32),
                                in0=sps[:].bitcast(I32),
                                scalar1=0x7FFFFFFF, scalar2=None,
                                op0=OP.bitwise_and,
                            )
                    elif path == "dve":
                        for b in range(B_LOC):
                            nc.vector.tensor_scalar(
                                out=absd[:, b, :], in0=coords[:],
                                scalar1=pos_t[:, k, b, a:a + 1], scalar2=None,
                                op0=OP.subtract,
                            )
                        nc.vector.tensor_scalar(
                            out=absd[:].bitcast(I32),
                            in0=absd[:].bitcast(I32),
                            scalar1=0x7FFFFFFF, scalar2=None,
                            op0=OP.bitwise_and,
                        )
                    else:  # act
                        for b in range(B_LOC):
                            nc.scalar.activation(
                                out=absd[:, b, :], in_=coords[:], func=AF.Abs,
                                bias=pos_t[:, k, b, (a + 2):(a + 3)], scale=1.0,
                            )
                    nc.scalar.activation(
                        out=p[:], in_=absd[:], func=AF.Sigmoid,
                        bias=gs_t[:, k, a:a + 1], scale=-2.0,
                    )
                    prof[(k, a)] = p

                # wave-A splat accumulation for this chunk
                for b in range(WAVE):
                    for my in (0, 1):
                        nc.tensor.matmul(
                            dA[(b, my)][:],
                            prof[(k, 1)][:, b, my * 128:(my + 1) * 128],
                            prof[(k, 0)][:, b, :],
                            start=(k == 0), stop=(k == kch - 1),
                        )

            # wave-A PSUM -> SBUF (f32r) while sub pool is still open; the
            # copies only depend on dA, scheduler orders them after stop MMs
            for b in range(WAVE):
                for my in (0, 1):
                    nc.vector.tensor_copy(d_sbA[b][:, my, :], dA[(b, my)][:])

        def smooth_and_loss(b, d_sb, ps_w, ps_z):
            w_sb = sbwork.tile([128, 2, GRID], F32R, tag="w_sb")
            wm = ps_w.tile([128, 2, GRID], F32, tag="w")
            for mx in (0, 1):
                for my in (0, 1):
                    nc.tensor.matmul(
                        wm[:, mx, :],
                        d_sb[:, my, mx * 128:(mx + 1) * 128],
                        smt[:, my, :],
                        start=(my == 0), stop=(my == 1),
                    )
            nc.scalar.copy(w_sb[:], wm[:])

            zm = ps_z.tile([128, 2, GRID], F32, tag="z")
            for ma in (0, 1):
                for mx in (0, 1):
                    nc.tensor.matmul(
                        zm[:, ma, :],
                        w_sb[:, mx, ma * 128:(ma + 1) * 128],
                        smt[:, mx, :],
                        start=(mx == 0), stop=(mx == 1),
                    )
            z_sb = sbwork.tile([128, 2, GRID], F32, tag="z_sb")
            nc.vector.tensor_copy(z_sb[:], zm[:])
            nc.sync.dma_start(
                dens_ap[b].rearrange("(m p) g -> p m g", p=128), z_sb[:])

            t = scratch.tile([128, 2, GRID], F32, tag="relu")
            nc.vector.tensor_scalar(
                out=t[:], in0=z_sb[:], scalar1=1.0, scalar2=0.0,
                op0=OP.subtract, op1=OP.max,
            )
            nc.vector.scalar_tensor_tensor(
                out=t[:], in0=t[:], scalar=0.0, in1=t[:],
                op0=OP.add, op1=OP.mult,
                accum_out=lacc[:, b:b + 1],
            )

        with (
            tc.tile_pool(name="pdB", bufs=2, space="PSUM") as ps_dB,
            tc.tile_pool(name="pw", bufs=2, space="PSUM") as ps_w,
            tc.tile_pool(name="pz", bufs=2, space="PSUM") as ps_z,
        ):
            # wave-A smoothing + wave-B splat interleave via scheduler
            for b in range(WAVE):
                smooth_and_loss(b, d_sbA[b], ps_w, ps_z)
            for b in range(WAVE, B_LOC):
                dB = ps_dB.tile([128, 2, GRID], F32, tag="dB")
                for my in (0, 1):
                    for k in range(kch):
                        nc.tensor.matmul(
                            dB[:, my, :],
                            prof[(k, 1)][:, b, my * 128:(my + 1) * 128],
                            prof[(k, 0)][:, b, :],
                            start=(k == 0), stop=(k == kch - 1),
                        )
                d_sb = sbwork.tile([128, 2, GRID], F32R, tag="d_sbB")
                nc.vector.tensor_copy(d_sb[:], dB[:])
                smooth_and_loss(b, d_sb, ps_w, ps_z)

        nc.sync.dma_start(lossp_d[:], lacc[:])

    nc.compile()
    return nc


def _prepare(positions, sizes, macro_mask):
    """Host-side sharding/compaction. Returns (kch, in_maps)."""
    positions = np.asarray(positions, dtype=np.float32)
    sizes = np.asarray(sizes, dtype=np.float32)
    macro_mask = np.asarray(macro_mask)

    G = GRID
    keep = np.nonzero(macro_mask)[0]
    vk = len(keep)
    kch = max(1, (vk + 127) // 128)
    vp = kch * 128

    grid_pos = ((positions + np.float32(1.0)) / np.float32(2.0)
                * np.float32(G - 1)).astype(np.float32)
    grid_sizes = (sizes * np.float32(G / 2.0)).astype(np.float32)

    pos_c = np.full((B, vp, 2), PAD_POS, dtype=np.float32)
    pos_c[:, :vk, :] = grid_pos[:, keep, :]
    gs_c = np.zeros((vp, 2), dtype=np.float32)
    gs_c[:vk, :] = grid_sizes[keep, :]

    pos_r = pos_c.reshape(B, kch, 128, 2)                       # [b,k,v,a]
    gs_r = gs_c.reshape(kch, 128, 2).transpose(1, 0, 2).copy()  # [v,k,a]

    coords_np = np.tile(np.arange(G, dtype=np.float32), (128, 1)).copy()
    smt_np = np.ascontiguousarray(_smoothing_matrix().T.reshape(2, 128, G))

    # fp16 exact-split subtract operands
    p_hi = pos_r.astype(np.float16)                             # [b,k,v,a]
    p_lo = (pos_r - p_hi.astype(np.float32)).astype(np.float16)
    cr_np = np.stack([np.arange(G, dtype=np.float16),
                      np.ones(G, np.float16), np.ones(G, np.float16)])

    in_maps = []
    for c in range(NCORES):
        bsl = slice(c * B_LOC, (c + 1) * B_LOC)
        pc = pos_r[bsl].transpose(2, 1, 0, 3)                   # (128,kch,B_LOC,2)
        pos_core = np.concatenate([pc, -pc], axis=3)
        ones = np.ones((1, kch, B_LOC, 2, 128), np.float16)
        # [b,k,v,a] -> [k,b,a,v]
        posl_core = np.concatenate([
            ones,
            -p_hi[bsl].transpose(1, 0, 3, 2)[None],
            -p_lo[bsl].transpose(1, 0, 3, 2)[None],
        ], axis=0)
        in_maps.append({
            "pos": np.ascontiguousarray(pos_core),
            "gs": gs_r,
            "coords": coords_np,
            "smt": smt_np,
            "posl": np.ascontiguousarray(posl_core.astype(np.float16)),
            "cr": np.ascontiguousarray(cr_np),
        })
    return kch, in_maps


def _postprocess(results):
    G = GRID
    dens = np.concatenate([r["dens"] for r in results], axis=0)
    density = dens[:, None, :, :].astype(np.float32)
    loss_total = np.sum(
        [r["lossp"].astype(np.float64).sum() for r in results])
    overflow_loss = np.float32(loss_total / (B * G * G))
    return density, overflow_loss


def get_program(kch):
    if kch not in _prog_cache:
        _prog_cache[kch] = _build_program(kch)
    return _prog_cache[kch]


def kernel(positions, sizes, macro_mask):
    from concourse.bass_utils import run_bass_kernel_spmd

    kch, in_maps = _prepare(positions, sizes, macro_mask)
    nc = get_program(kch)
    res = run_bass_kernel_spmd(nc, in_maps, core_ids=list(range(NCORES)))
    return _postprocess(res.results)
